# revision 31
# baseline (speedup 1.0000x reference)
"""Trainium2 Bass kernel for nn_AdaptiveActivationBlock (grouped deformable
conv block: offset conv -> affine-grid bilinear deform conv -> BN -> residual
ReLU).

Strategy v2 (8 NeuronCores, SPMD, zero collectives):
  - Affine grid folded into offset-conv weights on host; PE produces per-tap
    offsets for both directions in one 72-row PSUM set (halves offset-conv
    PE time vs per-direction sets).
  - Bilinear weights are tents of the clamped offsets; tents computed once on
    72 rows in bf16, x-tents DMA-aligned onto the y rows, then 9 (dy,dx)
    products on 36-row tiles; round-trip through DRAM broadcasts u to 128
    partitions (i-major rows i*4+b so each per-b sub-DMA spans stride-4
    partitions across all 16 SDMA engines).
  - INPUT-side sampling: tmp[k,d] = u[k,d] o xslab-shift, then the deform
    weights W_k (BN-folded) contract AND accumulate all 81 terms directly in
    PSUM (no F production, no identity-matmul accumulate).
  - Per tap the 9 products are 2 DVE ops (multi-dim APs, dx-parity split via
    an odd-shifted slab copy) to amortize per-op overhead at 2x bf16 mode.
  - Residual enters PSUM via one eye-matmul set; epilogue is ACT
    Relu(psum + bias) straight to bf16.
  - Sharding: (group, H-octant) blocks; 136 blocks / 8 cores = 17 each.
"""
import numpy as np
from ml_dtypes import bfloat16

import concourse.bass as bass
import concourse.tile as tile
from concourse.tile_rust import add_dep_helper
from concourse import bacc, mybir
from concourse.bass_utils import run_bass_kernel_spmd

G = 17
B = 4
CG = 32
H = W = 96
BN_EPS = 1e-5
CLAMP = 0.999
NB = 17            # blocks per core
ROWS = 12          # output rows per block
GR, GC = 16, 100   # padded slab grid
NGRID = GR * GC
NOUT = ROWS * W
KY = [-1, -1, -1, 0, 0, 0, 1, 1, 1]
KX = [-1, 0, 1, -1, 0, 1, -1, 0, 1]
F32 = mybir.dt.float32
BF16 = mybir.dt.bfloat16
ALU = mybir.AluOpType
ACTF = mybir.ActivationFunctionType

# d-block order: A-group = dx in {-1,+1} (dy-major), B-group = dx == 0
DORD = [(-1, -1), (-1, 1), (0, -1), (0, 1), (1, -1), (1, 1),
        (-1, 0), (0, 0), (1, 0)]

# core c gets all 8 octants of g=2c and g=2c+1, plus octant c of g=16,
# so weight tiles change only at j in {0, 8, 16} uniformly across cores
BLOCKS = []
for _c in range(8):
    BLOCKS += [(2 * _c, o) for o in range(8)]
    BLOCKS += [(2 * _c + 1, o) for o in range(8)]
    BLOCKS.append((16, _c))

_nc_cache = None


def _prep_weights(w_off, b_off, w_def, gamma, beta, run_mean, run_var):
    ky = np.array(KY, np.float32)
    kx = np.array(KX, np.float32)
    w_off = w_off.reshape(G, 2, 3, CG, 3, 3)
    b_off = b_off.reshape(G, 2, 3)
    wofk = (ky[None, None, :, None, None, None] * w_off[:, :, 0][:, :, None]
            + kx[None, None, :, None, None, None] * w_off[:, :, 1][:, :, None]
            + w_off[:, :, 2][:, :, None])               # [G,2,9,CG,3,3]
    bofk = (ky[None, None, :] * b_off[:, :, 0:1]
            + kx[None, None, :] * b_off[:, :, 1:2]
            + b_off[:, :, 2:3])                          # [G,2,9]
    s = gamma / np.sqrt(run_var + BN_EPS)
    t = beta - run_mean * s
    wdef = w_def.reshape(G, CG, CG, 3, 3) * s.reshape(G, CG, 1, 1, 1)
    wdef = wdef.reshape(G, CG, CG, 9)                    # [G,o,i,k]
    return wofk, bofk, wdef, t.reshape(G, CG)


def _host_pack(x, wofk, bofk, wdef, tbias):
    """Build the 8 per-core input dicts.

    Row layouts: contraction rows (xslab, wdefk rows, woffm rows) are i-major
    (p = i*4+b) so the per-b u-broadcast lands on stride-4 partitions;
    output rows (acc/residual/out, wdefk cols) are o-major (p = o*4+b)."""
    maps = []
    ii = np.arange(CG)
    for c in range(8):
        blks = BLOCKS[c * NB:(c + 1) * NB]
        xslab = np.zeros((NB, 128, NGRID), np.float32)
        wdefk = np.zeros((3, 128, 9, 128), np.float32)
        woffm = np.zeros((3, 128, 9, 72), np.float32)
        bt = np.zeros((NB, 128, 2), np.float32)
        for j, (g, oc) in enumerate(blks):
            r0 = 12 * oc
            xg = x[:, g * CG:(g + 1) * CG]              # [B,32,96,96]
            slab = np.zeros((B, CG, GR, GC), np.float32)
            rlo, rhi = max(r0 - 2, 0), min(r0 + 14, H)
            slab[:, :, rlo - (r0 - 2):rlo - (r0 - 2) + (rhi - rlo), 2:98] = \
                xg[:, :, rlo:rhi]
            # i-major rows: row i*4+b
            xslab[j] = slab.transpose(1, 0, 2, 3).reshape(128, NGRID)
            # W_k lhsT[(i*4+b), k, (o*4+b)] = wdef[g][o,i,k]; slot j//8
            wd_full = np.zeros((CG, B, 9, CG, B), np.float32)
            for b in range(B):
                wd_full[:, b, :, :, b] = wdef[g].transpose(1, 2, 0)  # [i,k,o]
            wdefk[j // 8] = wd_full.reshape(128, 9, 128)
            for b in range(B):
                for r in range(2):
                    # offset conv lhsT: col r*36+b*9+k, stencil tap p:
                    # woffm[slot, i*4+b, p, col] = wofk[g,r,k,i,py,px]
                    woffm[j // 8, ii * 4 + b, :, r * 36 + b * 9:r * 36 + b * 9 + 9] = \
                        wofk[g, r].transpose(1, 0, 2, 3).reshape(CG, 9, 9).transpose(0, 2, 1)
                    bt[j, r * 36 + b * 9:r * 36 + b * 9 + 9, 0] = bofk[g, r]
            bt[j, :, 1] = np.repeat(tbias[g], 4)         # row o*4+b -> t[o]
        maps.append(dict(
            xslab=xslab.astype(bfloat16),
            wdefk=wdefk.reshape(3, 128, 9 * 128).astype(bfloat16),
            woffm=woffm.reshape(3, 128, 9 * 72).astype(bfloat16),
            bt=bt,
            eye=np.eye(128, dtype=bfloat16)))
    return maps


def _build_nc():
    nc = bacc.Bacc(None, target_bir_lowering=False)
    d_xs = nc.dram_tensor("xslab", [NB, 128, NGRID], BF16, kind="ExternalInput")
    d_wd = nc.dram_tensor("wdefk", [3, 128, 9 * 128], BF16, kind="ExternalInput")
    d_wo = nc.dram_tensor("woffm", [3, 128, 9 * 72], BF16, kind="ExternalInput")
    d_bt = nc.dram_tensor("bt", [NB, 128, 2], F32, kind="ExternalInput")
    d_eye = nc.dram_tensor("eye", [128, 128], BF16, kind="ExternalInput")
    d_out = nc.dram_tensor("out", [NB, 128, NOUT], BF16, kind="ExternalOutput")

    with tile.TileContext(nc) as tc:
        with (
            tc.tile_pool(name="xs", bufs=2) as p_xs,
            tc.tile_pool(name="wts", bufs=2) as p_w,
            tc.tile_pool(name="sm", bufs=2) as p_sm,
            tc.tile_pool(name="tents", bufs=2) as p_t,
            tc.tile_pool(name="um", bufs=2) as p_um,
            tc.tile_pool(name="urep", bufs=3) as p_ur,
            tc.tile_pool(name="tmp", bufs=2) as p_tmp,
            tc.tile_pool(name="oo", bufs=2) as p_o,
            tc.tile_pool(name="eye", bufs=1) as p_eye,
            tc.tile_pool(name="umd", bufs=3, space="DRAM") as p_umd,
            tc.tile_pool(name="psacc", bufs=2, space=bass.MemorySpace.PSUM) as p_pa,
            tc.tile_pool(name="psoff", bufs=2, space=bass.MemorySpace.PSUM) as p_po,
        ):
            eye = p_eye.tile([128, 128], BF16)
            nc.sync.dma_start(eye[:], d_eye[:])
            umd_reads = {0: [], 1: [], 2: []}
            umd_write = {0: None, 1: None, 2: None}

            cur_w = {}

            def phase_off(j):
                """IO + offset conv + tents (products/umd-write deferred)."""
                xs = p_xs.tile([128, NGRID], BF16, tag="xs", name="xs")
                nc.sync.dma_start(xs[:], d_xs[j])
                if j % 8 == 0:
                    wdk = p_w.tile([128, 9 * 128], BF16, tag="wdk")
                    nc.sync.dma_start(wdk[:], d_wd[j // 8])
                    wof = p_w.tile([128, 9 * 72], BF16, tag="wof")
                    nc.sync.dma_start(wof[:], d_wo[j // 8])
                    cur_w["wdk"], cur_w["wof"] = wdk, wof
                wdk, wof = cur_w["wdk"], cur_w["wof"]
                bt = p_sm.tile([128, 2], F32, tag="bt")
                nc.sync.dma_start(bt[:], d_bt[j])
                bo = bt[0:72, 0:1]
                tb = bt[0:128, 1:2]

                # odd-shifted slab copy (for dx-parity-odd views)
                xso = p_xs.tile([128, NGRID], BF16, tag="xso", name="xso")
                nc.scalar.copy(xso[:, 0:NGRID - 1], xs[:, 1:NGRID])

                xs_ap = xs[:]
                xs_pstep = xs_ap.ap[0][0]

                def xs_view(base, nrows, ncols=96):
                    return bass.AP(xs.tensor, xs_ap.offset + base,
                                   [[xs_pstep, 128], [GC, nrows], [1, ncols]])

                # ---- offset conv: [72, 384] psum chunks x 3 ----
                offb = p_t.tile([72, NOUT], BF16, tag="offb")
                for ch in range(3):
                    pso = p_po.tile([128, 384], F32, name=f"pso{ch}", tag="pst")
                    for p in range(9):
                        py, px = p // 3 - 1, p % 3 - 1
                        rhs = xs_view((2 + py + ch * 4) * GC + 2 + px, 4)
                        nc.tensor.matmul(
                            pso[0:72, :], wof[:, p * 72:p * 72 + 72],
                            rhs, start=(p == 0), stop=(p == 8))
                    nc.scalar.activation(offb[:, ch * 384:(ch + 1) * 384],
                                         pso[0:72, :],
                                         ACTF.Identity, bias=bo, scale=1.0)
                # ---- clamp + tents (both directions at once, bf16) ----
                # t6 cols: [un | u0 | up], rows 0:36 y-dir, 36:72 x-dir
                nc.vector.tensor_scalar(offb[:], offb[:], -CLAMP, CLAMP,
                                        ALU.max, ALU.min)
                t6 = p_t.tile([72, 3 * NOUT], BF16, tag="t6")
                nc.vector.tensor_scalar(t6[:, 2 * NOUT:3 * NOUT], offb[:],
                                        0.0, None, ALU.max)
                nc.vector.tensor_scalar(t6[:, 0:NOUT], offb[:],
                                        -1.0, 0.0, ALU.mult, ALU.max)
                nc.vector.tensor_tensor(t6[:, NOUT:2 * NOUT],
                                        t6[:, 2 * NOUT:3 * NOUT],
                                        t6[:, 0:NOUT], ALU.add)
                nc.vector.tensor_scalar(t6[:, NOUT:2 * NOUT],
                                        t6[:, NOUT:2 * NOUT],
                                        -1.0, 1.0, ALU.mult, ALU.add)
                return dict(xs=xs, xso=xso, wdk=wdk, tb=tb, t6=t6,
                            j=j, urq=None)

            def phase_prod(st):
                """x-tent align + u-products (emitted mid-previous-block so
                the scalar ring never head-of-line stalls on the tents)."""
                t6 = st["t6"]
                tx3 = p_t.tile([36, 3 * NOUT], BF16, tag="tx3")
                nc.scalar.dma_start(tx3[:], t6[36:72, :])
                umt = p_um.tile([36, 9 * NOUT], BF16, tag="umt")
                for di, (dy, dx) in enumerate(DORD):
                    nc.vector.tensor_tensor(
                        umt[:, di * NOUT:(di + 1) * NOUT],
                        t6[0:36, (dy + 1) * NOUT:(dy + 2) * NOUT],
                        tx3[:, (dx + 1) * NOUT:(dx + 2) * NOUT], ALU.mult)
                st["umt"] = umt

            def phase_wr(st):
                """umd DRAM write (emitted after its data is already ready)."""
                umd = p_umd.tile([36, 9 * NOUT], BF16)
                slot = st["j"] % 3
                wr = nc.sync.dma_start(umd[:], st["umt"][:])
                for rd in umd_reads[slot]:
                    add_dep_helper(wr.ins, rd.ins, reason="umd WAR")
                umd_reads[slot] = []
                umd_write[slot] = wr
                st["umd"] = umd
                st["slot"] = slot

            def bcast_u(k, umd, slot):
                umd_ap = umd[:]
                umd_rowstep = umd_ap.ap[0][0]
                ur = p_ur.tile([128, 9 * NOUT], BF16, name="ur", tag="ur")
                ur_ap = ur[:]
                ur_pstep = ur_ap.ap[0][0]
                for b in range(4):
                    src = bass.AP(umd.tensor,
                                  umd_ap.offset + (b * 9 + k) * umd_rowstep,
                                  [[0, 32], [1, 9 * NOUT]])
                    dst = bass.AP(ur.tensor, ur_ap.offset + b * ur_pstep,
                                  [[4 * ur_pstep, 32], [1, 9 * NOUT]])
                    eng = nc.sync if (k + b) % 2 == 0 else nc.scalar
                    rd = eng.dma_start(dst, src)
                    add_dep_helper(rd.ins, umd_write[slot].ins, reason="umd RAW")
                    umd_reads[slot].append(rd)
                return ur

            def phase_main(j, st, nxt_st):
                xs, xso, wdk, tb, umd = (st["xs"], st["xso"], st["wdk"],
                                         st["tb"], st["umd"])
                slot = st["slot"]
                accs = [p_pa.tile([128, 384], F32, name=f"acc{c}", tag=f"acc{c}")
                        for c in range(3)]
                xs_pstep = xs[:].ap[0][0]

                urq = st["urq"] or [bcast_u(0, umd, slot), bcast_u(1, umd, slot)]
                npass = 0
                for k in range(9):
                    ur = urq.pop(0)
                    if k + 2 <= 8:
                        urq.append(bcast_u(k + 2, umd, slot))
                    if k == 1 and nxt_st is not None:
                        phase_prod(nxt_st)
                    if k == 3 and nxt_st is not None:
                        phase_wr(nxt_st)
                    if k == 5 and nxt_st is not None:
                        nxt_st["urq"] = [bcast_u(0, nxt_st["umd"], nxt_st["slot"])]
                    if k == 6 and nxt_st is not None:
                        nxt_st["urq"].append(
                            bcast_u(1, nxt_st["umd"], nxt_st["slot"]))
                    ur_ap = ur[:]
                    ur_pstep = ur_ap.ap[0][0]

                    # --- A-group: 6 blocks (dy in -1,0,1) x (dx in -1,+1),
                    # one op per dy (ISA allows 3 free dims) ---
                    tmpA = p_tmp.tile([128, 6 * NOUT], BF16, name="tmpA", tag="tmpA")
                    tA = tmpA[:]
                    for dyi in range(3):
                        baseA = (2 + KY[k] + dyi - 1) * GC + 2 + KX[k] - 1
                        srcA, offA = (xs, baseA) if baseA % 2 == 0 else (xso, baseA - 1)
                        sA = srcA[:]
                        zA = bass.AP(srcA.tensor, sA.offset + offA,
                                     [[xs_pstep, 128], [2, 2],
                                      [GC, ROWS], [1, 96]])
                        uA = bass.AP(ur.tensor, ur_ap.offset + 2 * dyi * NOUT,
                                     [[ur_pstep, 128], [NOUT, 2],
                                      [96, ROWS], [1, 96]])
                        outA = bass.AP(tmpA.tensor, tA.offset + 2 * dyi * NOUT,
                                       [[tA.ap[0][0], 128], [NOUT, 2],
                                        [96, ROWS], [1, 96]])
                        nc.vector.tensor_tensor(outA, zA, uA, ALU.mult)

                    # --- B-group: 3 blocks (dy in -1,0,1), dx == 0 ---
                    baseB = (2 + KY[k] - 1) * GC + 2 + KX[k]
                    srcB, offB = (xs, baseB) if baseB % 2 == 0 else (xso, baseB - 1)
                    sB = srcB[:]
                    zB = bass.AP(srcB.tensor, sB.offset + offB,
                                 [[xs_pstep, 128], [GC, 3], [GC, ROWS], [1, 96]])
                    uB = bass.AP(ur.tensor, ur_ap.offset + 6 * NOUT,
                                 [[ur_pstep, 128], [NOUT, 3], [96, ROWS], [1, 96]])
                    tmpB = p_tmp.tile([128, 3 * NOUT], BF16, name="tmpB", tag="tmpB")
                    tB = tmpB[:]
                    outB = bass.AP(tmpB.tensor, tB.offset,
                                   [[tB.ap[0][0], 128], [NOUT, 3],
                                    [96, ROWS], [1, 96]])
                    nc.vector.tensor_tensor(outB, zB, uB, ALU.mult)

                    # --- contraction+accumulate: 27 matmuls, lhsT = W_k ---
                    wdk_k = wdk[:, k * 128:(k + 1) * 128]
                    for di in range(9):
                        tsrc = tmpA if di < 6 else tmpB
                        doff = di * NOUT if di < 6 else (di - 6) * NOUT
                        for c in range(3):
                            nc.tensor.matmul(
                                accs[c][:], wdk_k,
                                tsrc[:, doff + c * 384:doff + (c + 1) * 384],
                                start=(npass == 0), stop=False,
                                skip_group_check=True)
                        npass += 1

                # ---- residual into psum + epilogue ----
                # residual = xslab rows 2..13, cols 2..97 (o*4+b == i*4+b)
                for c in range(3):
                    xrv = bass.AP(xs.tensor, xs[:].offset + (2 + 4 * c) * GC + 2,
                                  [[xs_pstep, 128], [GC, 4], [1, 96]])
                    nc.tensor.matmul(accs[c][:], eye[:], xrv,
                                     start=False, stop=True,
                                     skip_group_check=True)
                ot = p_o.tile([128, NOUT], BF16, tag="ot", name="ot")
                for c in range(3):
                    nc.scalar.activation(ot[:, c * 384:(c + 1) * 384], accs[c][:],
                                         ACTF.Relu, bias=tb, scale=1.0)
                    nc.scalar.dma_start(d_out[j, :, c * 384:(c + 1) * 384],
                                        ot[:, c * 384:(c + 1) * 384])

            st = phase_off(0)
            phase_prod(st)
            phase_wr(st)
            for j in range(NB):
                nxt_st = phase_off(j + 1) if j + 1 < NB else None
                phase_main(j, st, nxt_st)
                st = nxt_st
    nc.compile()
    return nc


def kernel(x, w_off, b_off, w_def, gamma, beta, run_mean, run_var):
    global _nc_cache
    x = np.ascontiguousarray(np.asarray(x, np.float32))
    wofk, bofk, wdef, tbias = _prep_weights(
        np.asarray(w_off, np.float32), np.asarray(b_off, np.float32),
        np.asarray(w_def, np.float32), np.asarray(gamma, np.float32),
        np.asarray(beta, np.float32), np.asarray(run_mean, np.float32),
        np.asarray(run_var, np.float32))
    in_maps = _host_pack(x, wofk, bofk, wdef, tbias)
    if _nc_cache is None:
        _nc_cache = _build_nc()
    res = run_bass_kernel_spmd(_nc_cache, in_maps, core_ids=list(range(8)))
    out = np.zeros((B, G * CG, H, W), np.float32)
    for c in range(8):
        # rows are o-major: p = o*4+b
        o = np.asarray(res.results[c]["out"]).astype(np.float32).reshape(NB, CG, B, ROWS, W)
        for j, (g, oc) in enumerate(BLOCKS[c * NB:(c + 1) * NB]):
            out[:, g * CG:(g + 1) * CG, 12 * oc:12 * oc + 12] = \
                o[j].transpose(1, 0, 2, 3)
    return out


# revision 33
# speedup vs baseline: 1.1074x; 1.1074x over previous
"""Trainium2 Bass kernel for nn_AdaptiveActivationBlock (grouped deformable
conv block: offset conv -> affine-grid bilinear deform conv -> BN -> residual
ReLU).

Strategy v2 (8 NeuronCores, SPMD, zero collectives):
  - Affine grid folded into offset-conv weights on host; PE produces per-tap
    offsets for both directions in one 72-row PSUM set (halves offset-conv
    PE time vs per-direction sets).
  - Bilinear weights are tents of the clamped offsets; tents computed once on
    72 rows in bf16, x-tents DMA-aligned onto the y rows, then 9 (dy,dx)
    products on 36-row tiles; round-trip through DRAM broadcasts u to 128
    partitions (i-major rows i*4+b so each per-b sub-DMA spans stride-4
    partitions across all 16 SDMA engines).
  - INPUT-side sampling: tmp[k,d] = u[k,d] o xslab-shift, then the deform
    weights W_k (BN-folded) contract AND accumulate all 81 terms directly in
    PSUM (no F production, no identity-matmul accumulate).
  - Per tap the 9 products are 2 DVE ops (multi-dim APs, dx-parity split via
    an odd-shifted slab copy) to amortize per-op overhead at 2x bf16 mode.
  - Residual enters PSUM via one eye-matmul set; epilogue is ACT
    Relu(psum + bias) straight to bf16.
  - Sharding: (group, H-octant) blocks; 136 blocks / 8 cores = 17 each.
"""
import numpy as np
from ml_dtypes import bfloat16

import concourse.bass as bass
import concourse.tile as tile
from concourse.tile_rust import add_dep_helper
from concourse import bacc, mybir
from concourse.bass_utils import run_bass_kernel_spmd

G = 17
B = 4
CG = 32
H = W = 96
BN_EPS = 1e-5
CLAMP = 0.999
NB = 17            # blocks per core
ROWS = 12          # output rows per block
GR, GC = 16, 100   # padded slab grid
NGRID = GR * GC
NOUT = ROWS * W
KY = [-1, -1, -1, 0, 0, 0, 1, 1, 1]
KX = [-1, 0, 1, -1, 0, 1, -1, 0, 1]
F32 = mybir.dt.float32
BF16 = mybir.dt.bfloat16
ALU = mybir.AluOpType
ACTF = mybir.ActivationFunctionType

# d-block order: A-group = dx in {-1,+1} (dy-major), B-group = dx == 0
DORD = [(-1, -1), (-1, 1), (0, -1), (0, 1), (1, -1), (1, 1),
        (-1, 0), (0, 0), (1, 0)]

# core c gets all 8 octants of g=2c and g=2c+1, plus octant c of g=16,
# so weight tiles change only at j in {0, 8, 16} uniformly across cores
BLOCKS = []
for _c in range(8):
    BLOCKS += [(2 * _c, o) for o in range(8)]
    BLOCKS += [(2 * _c + 1, o) for o in range(8)]
    BLOCKS.append((16, _c))

_nc_cache = None


def _prep_weights(w_off, b_off, w_def, gamma, beta, run_mean, run_var):
    ky = np.array(KY, np.float32)
    kx = np.array(KX, np.float32)
    w_off = w_off.reshape(G, 2, 3, CG, 3, 3)
    b_off = b_off.reshape(G, 2, 3)
    wofk = (ky[None, None, :, None, None, None] * w_off[:, :, 0][:, :, None]
            + kx[None, None, :, None, None, None] * w_off[:, :, 1][:, :, None]
            + w_off[:, :, 2][:, :, None])               # [G,2,9,CG,3,3]
    bofk = (ky[None, None, :] * b_off[:, :, 0:1]
            + kx[None, None, :] * b_off[:, :, 1:2]
            + b_off[:, :, 2:3])                          # [G,2,9]
    s = gamma / np.sqrt(run_var + BN_EPS)
    t = beta - run_mean * s
    wdef = w_def.reshape(G, CG, CG, 3, 3) * s.reshape(G, CG, 1, 1, 1)
    wdef = wdef.reshape(G, CG, CG, 9)                    # [G,o,i,k]
    return wofk, bofk, wdef, t.reshape(G, CG)


def _host_pack(x, wofk, bofk, wdef, tbias):
    """Build the 8 per-core input dicts.

    Row layouts: contraction rows (xslab, wdefk rows, woffm rows) are i-major
    (p = i*4+b) so the per-b u-broadcast lands on stride-4 partitions;
    output rows (acc/residual/out, wdefk cols) are o-major (p = o*4+b)."""
    maps = []
    ii = np.arange(CG)
    for c in range(8):
        blks = BLOCKS[c * NB:(c + 1) * NB]
        xslab = np.zeros((NB, 128, NGRID), np.float32)
        wdefk = np.zeros((3, 128, 9, 128), np.float32)
        woffm = np.zeros((3, 128, 9, 72), np.float32)
        bt = np.zeros((NB, 128, 2), np.float32)
        for j, (g, oc) in enumerate(blks):
            r0 = 12 * oc
            xg = x[:, g * CG:(g + 1) * CG]              # [B,32,96,96]
            slab = np.zeros((B, CG, GR, GC), np.float32)
            rlo, rhi = max(r0 - 2, 0), min(r0 + 14, H)
            slab[:, :, rlo - (r0 - 2):rlo - (r0 - 2) + (rhi - rlo), 2:98] = \
                xg[:, :, rlo:rhi]
            # i-major rows: row i*4+b
            xslab[j] = slab.transpose(1, 0, 2, 3).reshape(128, NGRID)
            # W_k lhsT[(i*4+b), k, (o*4+b)] = wdef[g][o,i,k]; slot j//8
            wd_full = np.zeros((CG, B, 9, CG, B), np.float32)
            for b in range(B):
                wd_full[:, b, :, :, b] = wdef[g].transpose(1, 2, 0)  # [i,k,o]
            wdefk[j // 8] = wd_full.reshape(128, 9, 128)
            for b in range(B):
                for r in range(2):
                    # offset conv lhsT: col r*36+b*9+k, stencil tap p:
                    # woffm[slot, i*4+b, p, col] = wofk[g,r,k,i,py,px]
                    woffm[j // 8, ii * 4 + b, :, r * 36 + b * 9:r * 36 + b * 9 + 9] = \
                        wofk[g, r].transpose(1, 0, 2, 3).reshape(CG, 9, 9).transpose(0, 2, 1)
                    bt[j, r * 36 + b * 9:r * 36 + b * 9 + 9, 0] = bofk[g, r]
            bt[j, :, 1] = np.repeat(tbias[g], 4)         # row o*4+b -> t[o]
        maps.append(dict(
            xslab=xslab.astype(bfloat16),
            wdefk=wdefk.reshape(3, 128, 9 * 128).astype(bfloat16),
            woffm=woffm.reshape(3, 128, 9 * 72).astype(bfloat16),
            bt=bt,
            eye=np.eye(128, dtype=bfloat16)))
    return maps


def _build_nc():
    nc = bacc.Bacc(None, target_bir_lowering=False)
    d_xs = nc.dram_tensor("xslab", [NB, 128, NGRID], BF16, kind="ExternalInput")
    d_wd = nc.dram_tensor("wdefk", [3, 128, 9 * 128], BF16, kind="ExternalInput")
    d_wo = nc.dram_tensor("woffm", [3, 128, 9 * 72], BF16, kind="ExternalInput")
    d_bt = nc.dram_tensor("bt", [NB, 128, 2], F32, kind="ExternalInput")
    d_eye = nc.dram_tensor("eye", [128, 128], BF16, kind="ExternalInput")
    d_out = nc.dram_tensor("out", [NB, 128, NOUT], BF16, kind="ExternalOutput")

    with tile.TileContext(nc) as tc:
        with (
            tc.tile_pool(name="xs", bufs=2) as p_xs,
            tc.tile_pool(name="wts", bufs=2) as p_w,
            tc.tile_pool(name="sm", bufs=2) as p_sm,
            tc.tile_pool(name="tents", bufs=2) as p_t,
            tc.tile_pool(name="um", bufs=2) as p_um,
            tc.tile_pool(name="urep", bufs=3) as p_ur,
            tc.tile_pool(name="tmp", bufs=2) as p_tmp,
            tc.tile_pool(name="oo", bufs=2) as p_o,
            tc.tile_pool(name="eye", bufs=1) as p_eye,
            tc.tile_pool(name="umd", bufs=3, space="DRAM") as p_umd,
            tc.tile_pool(name="psacc", bufs=2, space=bass.MemorySpace.PSUM) as p_pa,
            tc.tile_pool(name="psoff", bufs=2, space=bass.MemorySpace.PSUM) as p_po,
        ):
            eye = p_eye.tile([128, 128], BF16)
            nc.sync.dma_start(eye[:], d_eye[:])
            umd_reads = {0: [], 1: [], 2: []}
            umd_write = {0: None, 1: None, 2: None}

            cur_w = {}

            def phase_off(j):
                """IO + offset conv + tents (products/umd-write deferred)."""
                xs = p_xs.tile([128, NGRID], BF16, tag="xs", name="xs")
                nc.sync.dma_start(xs[:], d_xs[j])
                if j % 8 == 0:
                    wdk = p_w.tile([128, 9 * 128], BF16, tag="wdk")
                    nc.sync.dma_start(wdk[:], d_wd[j // 8])
                    wof = p_w.tile([128, 9 * 72], BF16, tag="wof")
                    nc.sync.dma_start(wof[:], d_wo[j // 8])
                    cur_w["wdk"], cur_w["wof"] = wdk, wof
                wdk, wof = cur_w["wdk"], cur_w["wof"]
                bt = p_sm.tile([128, 2], F32, tag="bt")
                nc.sync.dma_start(bt[:], d_bt[j])
                bo = bt[0:72, 0:1]
                tb = bt[0:128, 1:2]

                # odd-shifted slab copy (for dx-parity-odd views)
                xso = p_xs.tile([128, NGRID], BF16, tag="xso", name="xso")
                nc.scalar.copy(xso[:, 0:NGRID - 1], xs[:, 1:NGRID])

                xs_ap = xs[:]
                xs_pstep = xs_ap.ap[0][0]

                def xs_view(base, nrows, ncols=96):
                    return bass.AP(xs.tensor, xs_ap.offset + base,
                                   [[xs_pstep, 128], [GC, nrows], [1, ncols]])

                # ---- offset conv: [72, 384] psum chunks x 3 ----
                offb = p_t.tile([72, NOUT], BF16, tag="offb")
                for ch in range(3):
                    pso = p_po.tile([128, 384], F32, name=f"pso{ch}", tag="pst")
                    for p in range(9):
                        py, px = p // 3 - 1, p % 3 - 1
                        rhs = xs_view((2 + py + ch * 4) * GC + 2 + px, 4)
                        nc.tensor.matmul(
                            pso[0:72, :], wof[:, p * 72:p * 72 + 72],
                            rhs, start=(p == 0), stop=(p == 8))
                    nc.scalar.activation(offb[:, ch * 384:(ch + 1) * 384],
                                         pso[0:72, :],
                                         ACTF.Identity, bias=bo, scale=1.0)
                # ---- clamp + tents (both directions at once, bf16) ----
                # t6 cols: [un | u0 | up], rows 0:36 y-dir, 36:72 x-dir
                nc.vector.tensor_scalar(offb[:], offb[:], -CLAMP, CLAMP,
                                        ALU.max, ALU.min)
                t6 = p_t.tile([72, 3 * NOUT], BF16, tag="t6")
                nc.vector.tensor_scalar(t6[:, 2 * NOUT:3 * NOUT], offb[:],
                                        0.0, None, ALU.max)
                nc.vector.tensor_scalar(t6[:, 0:NOUT], offb[:],
                                        -1.0, 0.0, ALU.mult, ALU.max)
                nc.vector.tensor_tensor(t6[:, NOUT:2 * NOUT],
                                        t6[:, 2 * NOUT:3 * NOUT],
                                        t6[:, 0:NOUT], ALU.add)
                nc.vector.tensor_scalar(t6[:, NOUT:2 * NOUT],
                                        t6[:, NOUT:2 * NOUT],
                                        -1.0, 1.0, ALU.mult, ALU.add)
                return dict(xs=xs, xso=xso, wdk=wdk, tb=tb, t6=t6,
                            j=j, urq=None)

            def phase_prod(st):
                """x-tent align + u-products (emitted mid-previous-block so
                the scalar ring never head-of-line stalls on the tents)."""
                t6 = st["t6"]
                tx3 = p_t.tile([36, 3 * NOUT], BF16, tag="tx3")
                nc.scalar.dma_start(tx3[:], t6[36:72, :])
                umt = p_um.tile([36, 9 * NOUT], BF16, tag="umt")
                for di, (dy, dx) in enumerate(DORD):
                    nc.vector.tensor_tensor(
                        umt[:, di * NOUT:(di + 1) * NOUT],
                        t6[0:36, (dy + 1) * NOUT:(dy + 2) * NOUT],
                        tx3[:, (dx + 1) * NOUT:(dx + 2) * NOUT], ALU.mult)
                st["umt"] = umt

            def phase_wr(st):
                """umd DRAM write (emitted after its data is already ready)."""
                umd = p_umd.tile([36, 9 * NOUT], BF16)
                slot = st["j"] % 3
                wr = nc.sync.dma_start(umd[:], st["umt"][:])
                for rd in umd_reads[slot]:
                    add_dep_helper(wr.ins, rd.ins, reason="umd WAR")
                umd_reads[slot] = []
                umd_write[slot] = wr
                st["umd"] = umd
                st["slot"] = slot

            def bcast_u(k, umd, slot):
                umd_ap = umd[:]
                umd_rowstep = umd_ap.ap[0][0]
                ur = p_ur.tile([128, 9 * NOUT], BF16, name="ur", tag="ur")
                ur_ap = ur[:]
                ur_pstep = ur_ap.ap[0][0]
                for b in range(4):
                    src = bass.AP(umd.tensor,
                                  umd_ap.offset + (b * 9 + k) * umd_rowstep,
                                  [[0, 32], [1, 9 * NOUT]])
                    dst = bass.AP(ur.tensor, ur_ap.offset + b * ur_pstep,
                                  [[4 * ur_pstep, 32], [1, 9 * NOUT]])
                    eng = nc.sync if (k + b) % 2 == 0 else nc.scalar
                    rd = eng.dma_start(dst, src)
                    add_dep_helper(rd.ins, umd_write[slot].ins, reason="umd RAW")
                    umd_reads[slot].append(rd)
                return ur

            def phase_main(j, st, nxt_st):
                xs, xso, wdk, tb, umd = (st["xs"], st["xso"], st["wdk"],
                                         st["tb"], st["umd"])
                slot = st["slot"]
                accs = [p_pa.tile([128, 384], F32, name=f"acc{c}", tag=f"acc{c}")
                        for c in range(3)]
                xs_pstep = xs[:].ap[0][0]

                urq = st["urq"] or [bcast_u(0, umd, slot), bcast_u(1, umd, slot)]
                npass = 0
                for k in range(9):
                    ur = urq.pop(0)
                    if k + 2 <= 8:
                        urq.append(bcast_u(k + 2, umd, slot))
                    if k == 1 and pending_out:
                        jj, oot = pending_out.pop()
                        nc.scalar.dma_start(d_out[jj], oot[:])
                    if k == 2 and nxt_st is not None:
                        phase_prod(nxt_st)
                    if k == 5 and nxt_st is not None:
                        phase_wr(nxt_st)
                    if k == 7 and nxt_st is not None:
                        nxt_st["urq"] = [bcast_u(0, nxt_st["umd"], nxt_st["slot"]),
                                         bcast_u(1, nxt_st["umd"], nxt_st["slot"])]
                    ur_ap = ur[:]
                    ur_pstep = ur_ap.ap[0][0]

                    # --- A-group: 6 blocks (dy in -1,0,1) x (dx in -1,+1),
                    # one op per dy (ISA allows 3 free dims) ---
                    tmpA = p_tmp.tile([128, 6 * NOUT], BF16, name="tmpA", tag="tmpA")
                    tA = tmpA[:]
                    for dyi in range(3):
                        baseA = (2 + KY[k] + dyi - 1) * GC + 2 + KX[k] - 1
                        srcA, offA = (xs, baseA) if baseA % 2 == 0 else (xso, baseA - 1)
                        sA = srcA[:]
                        zA = bass.AP(srcA.tensor, sA.offset + offA,
                                     [[xs_pstep, 128], [2, 2],
                                      [GC, ROWS], [1, 96]])
                        uA = bass.AP(ur.tensor, ur_ap.offset + 2 * dyi * NOUT,
                                     [[ur_pstep, 128], [NOUT, 2],
                                      [96, ROWS], [1, 96]])
                        outA = bass.AP(tmpA.tensor, tA.offset + 2 * dyi * NOUT,
                                       [[tA.ap[0][0], 128], [NOUT, 2],
                                        [96, ROWS], [1, 96]])
                        nc.vector.tensor_tensor(outA, zA, uA, ALU.mult)

                    # --- B-group: 3 blocks (dy in -1,0,1), dx == 0 ---
                    baseB = (2 + KY[k] - 1) * GC + 2 + KX[k]
                    srcB, offB = (xs, baseB) if baseB % 2 == 0 else (xso, baseB - 1)
                    sB = srcB[:]
                    zB = bass.AP(srcB.tensor, sB.offset + offB,
                                 [[xs_pstep, 128], [GC, 3], [GC, ROWS], [1, 96]])
                    uB = bass.AP(ur.tensor, ur_ap.offset + 6 * NOUT,
                                 [[ur_pstep, 128], [NOUT, 3], [96, ROWS], [1, 96]])
                    tmpB = p_tmp.tile([128, 3 * NOUT], BF16, name="tmpB", tag="tmpB")
                    tB = tmpB[:]
                    outB = bass.AP(tmpB.tensor, tB.offset,
                                   [[tB.ap[0][0], 128], [NOUT, 3],
                                    [96, ROWS], [1, 96]])
                    nc.vector.tensor_tensor(outB, zB, uB, ALU.mult)

                    # --- contraction+accumulate: 27 matmuls, lhsT = W_k ---
                    wdk_k = wdk[:, k * 128:(k + 1) * 128]
                    for di in range(9):
                        tsrc = tmpA if di < 6 else tmpB
                        doff = di * NOUT if di < 6 else (di - 6) * NOUT
                        for c in range(3):
                            nc.tensor.matmul(
                                accs[c][:], wdk_k,
                                tsrc[:, doff + c * 384:doff + (c + 1) * 384],
                                start=(npass == 0), stop=False,
                                skip_group_check=True)
                        npass += 1

                # ---- residual into psum + epilogue ----
                # residual = xslab rows 2..13, cols 2..97 (o*4+b == i*4+b)
                for c in range(3):
                    xrv = bass.AP(xs.tensor, xs[:].offset + (2 + 4 * c) * GC + 2,
                                  [[xs_pstep, 128], [GC, 4], [1, 96]])
                    nc.tensor.matmul(accs[c][:], eye[:], xrv,
                                     start=False, stop=True,
                                     skip_group_check=True)
                ot = p_o.tile([128, NOUT], BF16, tag="ot", name="ot")
                for c in range(3):
                    nc.scalar.activation(ot[:, c * 384:(c + 1) * 384], accs[c][:],
                                         ACTF.Relu, bias=tb, scale=1.0)
                if nxt_st is None:
                    nc.scalar.dma_start(d_out[j], ot[:])
                else:
                    pending_out.append((j, ot))

            pending_out = []
            st = phase_off(0)
            phase_prod(st)
            phase_wr(st)
            for j in range(NB):
                nxt_st = phase_off(j + 1) if j + 1 < NB else None
                phase_main(j, st, nxt_st)
                st = nxt_st
    nc.compile()
    return nc


def kernel(x, w_off, b_off, w_def, gamma, beta, run_mean, run_var):
    global _nc_cache
    x = np.ascontiguousarray(np.asarray(x, np.float32))
    wofk, bofk, wdef, tbias = _prep_weights(
        np.asarray(w_off, np.float32), np.asarray(b_off, np.float32),
        np.asarray(w_def, np.float32), np.asarray(gamma, np.float32),
        np.asarray(beta, np.float32), np.asarray(run_mean, np.float32),
        np.asarray(run_var, np.float32))
    in_maps = _host_pack(x, wofk, bofk, wdef, tbias)
    if _nc_cache is None:
        _nc_cache = _build_nc()
    res = run_bass_kernel_spmd(_nc_cache, in_maps, core_ids=list(range(8)))
    out = np.zeros((B, G * CG, H, W), np.float32)
    for c in range(8):
        # rows are o-major: p = o*4+b
        o = np.asarray(res.results[c]["out"]).astype(np.float32).reshape(NB, CG, B, ROWS, W)
        for j, (g, oc) in enumerate(BLOCKS[c * NB:(c + 1) * NB]):
            out[:, g * CG:(g + 1) * CG, 12 * oc:12 * oc + 12] = \
                o[j].transpose(1, 0, 2, 3)
    return out
